# revision 15
# baseline (speedup 1.0000x reference)
"""Trainium2 Bass kernel for AttentionPatcher (GQA attention block, S=2048).

Sharding: 8-way tensor parallel over KV head groups. Core c owns KV head c
and query heads 4c..4c+3: it computes its Q/K/V projections, RoPE, causal
attention, and a full partial o_proj (wo column shard); a ReduceScatter(add)
over the 8 cores then leaves core c with rows [512c, 512c+512) of the final
output, which the host concatenates.

All matmul operands are bf16 (fp32 PSUM accumulation). The PE only reaches
its top p-state after ~3us of continuous execution, so the kernel is
structured to keep the PE queue dense: all weights are SBUF-resident (wq/wo
loaded once up front on otherwise-idle DMA queues), and the attention inner
loop is software-pipelined — scores for pair i+1 issue before the AV/den
matmuls of pair i, so the PE never waits on the exp/mask chain.
"""
import os
import sys

import numpy as np
import ml_dtypes

if os.path.isdir("/opt/trn_rl_repo") and "/opt/trn_rl_repo" not in sys.path:
    sys.path.insert(0, "/opt/trn_rl_repo")

import concourse.bacc as bacc
import concourse.mybir as mybir
import concourse.tile as tile
from concourse.bass_utils import run_bass_kernel_spmd
from concourse.masks import make_identity

F32 = mybir.dt.float32
BF16 = mybir.dt.bfloat16
ActF = mybir.ActivationFunctionType
Alu = mybir.AluOpType
NP_BF16 = ml_dtypes.bfloat16

H, KV, D, S = 32, 8, 128, 2048
HID = H * D
NCORES = 8
G = H // KV          # query heads per core
ST = 512             # s-tile size
NST = S // ST        # 4 s-tiles
KO = HID // 128      # 32 contraction subtiles
MO = HID // 128      # 32 output row tiles
INV_SQRT_D = 1.0 / float(np.sqrt(D))


def build_nc(with_collective=True):
    nc = bacc.Bacc("TRN2", target_bir_lowering=False, debug=False)

    x = nc.dram_tensor("x", [KO, 128, S], BF16, kind="ExternalInput")
    wq = nc.dram_tensor("wq", [128, KO, G * 128], BF16, kind="ExternalInput")
    wk = nc.dram_tensor("wk", [128, KO, 128], BF16, kind="ExternalInput")
    wv = nc.dram_tensor("wv", [128, KO, 128], BF16, kind="ExternalInput")
    wo = nc.dram_tensor("wo", [128, MO, G, 128], BF16, kind="ExternalInput")
    bq = nc.dram_tensor("bq", [128, G], F32, kind="ExternalInput")
    bk = nc.dram_tensor("bk", [128, 1], F32, kind="ExternalInput")
    bv = nc.dram_tensor("bv", [128, 1], F32, kind="ExternalInput")
    cos = nc.dram_tensor("cos", [128, S], BF16, kind="ExternalInput")
    sin = nc.dram_tensor("sin", [128, S], BF16, kind="ExternalInput")
    rot = nc.dram_tensor("rot", [128, 128], BF16, kind="ExternalInput")
    yout = nc.dram_tensor("y", [G, 128, S], BF16, kind="ExternalOutput")

    with tile.TileContext(nc) as tc:
        with (
            tc.tile_pool(name="const", bufs=1) as const,
            tc.tile_pool(name="sb", bufs=3) as sb,
            tc.tile_pool(name="ps", bufs=1, space="PSUM") as ps,
            tc.tile_pool(name="dram", bufs=1, space="DRAM") as dram,
        ):
            # ---- resident weights / constants ----
            # wq/wk/wv go through the scalar+gpsimd DMA queues (chunked,
            # interleaved so early ko chunks land first) so the sync queue
            # starts streaming x immediately; wo is loaded later on the sync
            # queue, interleaved with si=1/2 x tiles.
            wq_sb = const.tile([128, KO, G * 128], BF16)
            wk_sb = const.tile([128, KO, 128], BF16)
            wv_sb = const.tile([128, KO, 128], BF16)
            wo_sb = const.tile([128, MO, G, 128], BF16)
            # fine-grained (2-ko) chunks: the first QKV matmuls only wait on
            # a small transfer, and the weight stream stays ahead of the PE
            for c16 in range(16):
                ksl = slice(c16 * 2, (c16 + 1) * 2)
                nc.scalar.dma_start(wq_sb[:, ksl, :], wq[:, ksl, :])
                nc.gpsimd.dma_start(wk_sb[:, ksl, :], wk[:, ksl, :])
                nc.gpsimd.dma_start(wv_sb[:, ksl, :], wv[:, ksl, :])
            cos_sb = const.tile([128, S], BF16)
            sin_sb = const.tile([128, S], BF16)
            nc.scalar.dma_start(cos_sb[:], cos[:, :])
            nc.gpsimd.dma_start(sin_sb[:], sin[:, :])
            rot_sb = const.tile([128, 128], BF16)
            nc.scalar.dma_start(rot_sb[:], rot[:, :])
            bq_sb = const.tile([128, G], F32)
            bk_sb = const.tile([128, 1], F32)
            bv_sb = const.tile([128, 1], F32)
            nc.scalar.dma_start(bq_sb[:], bq[:, :])
            nc.scalar.dma_start(bk_sb[:], bk[:, :])
            nc.scalar.dma_start(bv_sb[:], bv[:, :])
            ones_f = const.tile([128, 128], F32)
            nc.any.memset(ones_f[:], 1.0)
            ones_r = const.tile([128, 128], mybir.dt.float32r)
            nc.vector.tensor_copy(ones_r[:], ones_f[:])
            ident = const.tile([128, 128], F32)
            make_identity(nc, ident)

            # ---- resident activations ----
            k_rot = const.tile([128, S], BF16)          # K, (d, l) layout
            v_t = const.tile([128, S // 128, 128], BF16)  # V^T, (l%128, l//128, d)
            # attn out, one tile per (g, si) so o_proj deps stay per-slice
            out_t = [[const.tile([128, ST], BF16, name=f"out_{g}_{si}")
                      for si in range(NST)] for g in range(G)]

            def rope(raw_r, dst_ap, sl):
                """dst = raw*cos + (rot@raw)*sin over s-slice sl."""
                ps_rot = ps.tile([128, ST], F32, tag="mm", bufs=4, name="ps_rot")
                nc.tensor.matmul(ps_rot[:], rot_sb[:], raw_r[:],
                                 start=True, stop=True)
                t1 = sb.tile([128, ST], BF16, tag="rope_t1", bufs=2)
                t2 = sb.tile([128, ST], BF16, tag="rope_t2", bufs=2)
                nc.vector.tensor_tensor(t1[:], raw_r[:], cos_sb[:, sl], Alu.mult)
                nc.vector.tensor_tensor(t2[:], ps_rot[:], sin_sb[:, sl], Alu.mult)
                nc.vector.tensor_tensor(dst_ap, t1[:], t2[:], Alu.add)

            for si in range(NST):
                sl = slice(si * ST, (si + 1) * ST)
                # ---------- QKV projections for this s-tile ----------
                ps_q = [ps.tile([128, ST], F32, tag="mm", bufs=4, name=f"ps_q{g}")
                        for g in range(G)]
                ps_kv = ps.tile([128, 2 * ST], F32, tag="big", bufs=2)
                ps_k = ps_kv[:, 0:ST]
                ps_v = ps_kv[:, ST:2 * ST]
                # resident-wo chunk loads ride the scalar queue while the
                # scalar engine is idle (QKV of si=1/2, before exps start)
                if si in (1, 2):
                    for wc2 in range(4):
                        wc = (si - 1) * 4 + wc2
                        msl = slice(wc * (MO // 8), (wc + 1) * (MO // 8))
                        nc.scalar.dma_start(wo_sb[:, msl, :, :],
                                            wo[:, msl, :, :])
                for ko in range(KO):
                    xt = sb.tile([128, ST], BF16, tag="x", bufs=10)
                    nc.sync.dma_start(xt[:], x[ko][:, sl])
                    st = (ko == 0)
                    sp = (ko == KO - 1)
                    for g in range(G):
                        nc.tensor.matmul(ps_q[g][:],
                                         wq_sb[:, ko, g * 128:(g + 1) * 128],
                                         xt[:], start=st, stop=sp)
                    nc.tensor.matmul(ps_k, wk_sb[:, ko, :], xt[:],
                                     start=st, stop=sp)
                    nc.tensor.matmul(ps_v, wv_sb[:, ko, :], xt[:],
                                     start=st, stop=sp)

                nli = (si + 1) * (ST // 128)  # visible l-blocks
                npair = nli // 2

                # ---------- biases + rope, g0 chain first ----------
                # DVE order is chosen so q_rot[0] (which gates the first
                # scores matmul) is ready as early as possible; V work is
                # PE filler that overlaps the first exp.
                q_raws = []
                q_raw0 = sb.tile([128, ST], BF16, tag="q_raw", bufs=2,
                                 name="q_raw0")
                nc.vector.tensor_scalar(q_raw0[:], ps_q[0][:], bq_sb[:, 0:1],
                                        INV_SQRT_D, Alu.add, Alu.mult)
                v_sb = sb.tile([128, ST], F32, tag="v_sb", bufs=2)
                nc.vector.tensor_scalar(v_sb[:], ps_v, bv_sb[:, 0:1], None,
                                        Alu.add)
                q_rots = []
                q_rot0 = sb.tile([128, ST], BF16, tag="q_rot", bufs=4,
                                 name="q_rot0")
                rope(q_raw0, q_rot0[:], sl)
                q_rots.append(q_rot0)
                q_raw1 = sb.tile([128, ST], BF16, tag="q_raw", bufs=2,
                                 name="q_raw1")
                nc.vector.tensor_scalar(q_raw1[:], ps_q[1][:], bq_sb[:, 1:2],
                                        INV_SQRT_D, Alu.add, Alu.mult)
                q_rot1 = sb.tile([128, ST], BF16, tag="q_rot", bufs=4,
                                 name="q_rot1")
                rope(q_raw1, q_rot1[:], sl)
                q_rots.append(q_rot1)
                # V transpose into v_t: PE filler while the DVE works
                # through the rope chains
                for j in range(ST // 128):
                    ps_t = ps.tile([128, 128], F32, tag="mm", bufs=4)
                    nc.tensor.transpose(ps_t[:], v_sb[:, j * 128:(j + 1) * 128],
                                        ident[:])
                    nc.vector.tensor_copy(v_t[:, si * (ST // 128) + j, :],
                                          ps_t[:])
                k_raw = sb.tile([128, ST], BF16, tag="k_raw", bufs=2)
                nc.vector.tensor_scalar(k_raw[:], ps_k, bk_sb[:, 0:1], None,
                                        Alu.add)
                rope(k_raw, k_rot[:, sl], sl)
                for g in range(2, G):
                    q_raw = sb.tile([128, ST], BF16, tag="q_raw", bufs=2,
                                    name=f"q_raw{g}")
                    nc.vector.tensor_scalar(q_raw[:], ps_q[g][:],
                                            bq_sb[:, g:g + 1], INV_SQRT_D,
                                            Alu.add, Alu.mult)
                    q_rot = sb.tile([128, ST], BF16, tag="q_rot", bufs=4,
                                    name=f"q_rot{g}")
                    rope(q_raw, q_rot[:], sl)
                    q_rots.append(q_rot)

                # ---------- software-pipelined attention ----------
                jobs = [(g, pi) for g in range(G) for pi in range(npair)]

                def issue_scores(job):
                    g, pi = job
                    ps_s2 = ps.tile([128, 2 * ST], F32, tag="big", bufs=2)
                    offs = []
                    for h in range(2):
                        li = 2 * pi + h
                        j = li - si * (ST // 128)
                        # diagonal block j: columns [0, 128j) are fully
                        # masked -> skip them entirely
                        off = 128 * j if j > 0 else 0
                        offs.append(off)
                        nc.tensor.matmul(
                            ps_s2[:, h * ST + off:(h + 1) * ST],
                            k_rot[:, li * 128:(li + 1) * 128],
                            q_rots[g][:, off:], start=True, stop=True)
                    return ps_s2, offs

                cur = issue_scores(jobs[0])

                ps_av = {}
                ps_den = {}
                for idx, (g, pi) in enumerate(jobs):
                    ps_s2, offs = cur
                    p2 = sb.tile([128, 2 * ST], BF16, tag="p", bufs=4)
                    if offs == [0, 0]:
                        # off-diagonal pair: one contiguous exp (fewer
                        # instruction + semaphore hops on the scalar engine)
                        nc.scalar.activation(p2[:], ps_s2[:], ActF.Exp)
                    else:
                        for h in range(2):
                            off = offs[h]
                            nc.scalar.activation(
                                p2[:, h * ST + off:(h + 1) * ST],
                                ps_s2[:, h * ST + off:(h + 1) * ST],
                                ActF.Exp)
                    # keep the PE streaming: scores for the next pair go in
                    # front of this pair's AV/den in the PE queue
                    if idx + 1 < len(jobs):
                        nxt = issue_scores(jobs[idx + 1])
                    for h in range(2):
                        li = 2 * pi + h
                        j = li - si * (ST // 128)
                        if j >= 0:
                            off = offs[h]
                            # causal: keep where l <= s within the block
                            nc.gpsimd.affine_select(
                                out=p2[:, h * ST + off:(h + 1) * ST],
                                in_=p2[:, h * ST + off:(h + 1) * ST],
                                compare_op=Alu.is_ge, fill=0.0,
                                base=0, channel_multiplier=-1,
                                pattern=[[1, ST - off]],
                            )
                    if pi == 0:
                        ps_av[g] = ps.tile([128, ST], F32, tag="mm", bufs=4,
                                           name=f"ps_av{g}")
                        # denominator accumulates on the DVE (elementwise
                        # over l-blocks, partitions preserved); only one
                        # final ones-matmul per head does the partition sum.
                        # The chain has a full head of slack, so DVE latency
                        # never gates the PE.
                        ps_den[g] = sb.tile([128, ST], mybir.dt.float32r,
                                            tag="dacc", bufs=2,
                                            name=f"dacc{g}")
                    for h in range(2):
                        li = 2 * pi + h
                        off = offs[h]
                        ph = p2[:, h * ST + off:(h + 1) * ST]
                        nc.tensor.matmul(ps_av[g][:, off:], v_t[:, li, :], ph,
                                         start=(li == 0),
                                         stop=(li == nli - 1))
                        if li == 0:
                            nc.vector.tensor_copy(ps_den[g][:], ph)
                        else:
                            nc.vector.tensor_tensor(ps_den[g][:, off:],
                                                    ps_den[g][:, off:], ph,
                                                    Alu.add)
                    if idx + 1 < len(jobs):
                        cur = nxt
                    if pi == npair - 1:
                        # partition-sum of the accumulated p via one
                        # ones-matmul, then evict fast so PSUM slots recycle
                        ps_d = ps.tile([128, ST], F32, tag="mm", bufs=4,
                                       name="ps_d")
                        nc.tensor.matmul(ps_d[:], ones_r[:], ps_den[g][:],
                                         start=True, stop=True)
                        den_sb = sb.tile([128, ST], F32, tag="den_sb", bufs=2)
                        nc.vector.tensor_copy(den_sb[:], ps_d[:])
                        av_sb = sb.tile([128, ST], F32, tag="av_sb", bufs=2)
                        nc.scalar.activation(av_sb[:], ps_av[g][:], ActF.Copy)
                        recip = sb.tile([128, ST], F32, tag="recip", bufs=2)
                        nc.vector.reciprocal_approx_fast(recip[:], den_sb[:])
                        # normalize on the (otherwise idle) gpsimd engine so
                        # the DVE queue drains fast — the next s-tile's QKV
                        # psum slots wait on these evictions
                        nc.gpsimd.tensor_tensor(out_t[g][si][:], av_sb[:],
                                                recip[:], Alu.mult)

            # ---------- o_proj: y_partial = wo_colshard @ out ----------
            # chunked: after each group of 8 row-blocks, ReduceScatter that
            # chunk (overlaps the collective with the next group's compute)
            NCHUNK = G  # 4 chunks of 8 row-blocks
            MO_PER = MO // NCHUNK
            cc_in = dram.tile([MO, 128, S], BF16)
            cc_out = dram.tile([NCHUNK, 128, S], BF16)
            for chunk in range(NCHUNK):
                for mo in range(chunk * MO_PER, (chunk + 1) * MO_PER):
                    for si in range(NST):
                        ps_y = ps.tile([128, ST], F32, tag="mm", bufs=4)
                        for g in range(G):
                            nc.tensor.matmul(ps_y[:], wo_sb[:, mo, g, :],
                                             out_t[g][si][:],
                                             start=(g == 0), stop=(g == G - 1))
                        y_sb = sb.tile([128, ST], BF16, tag="y_sb", bufs=8)
                        dst = cc_in[mo][:, si * ST:(si + 1) * ST]
                        # evictions alternate scalar/vector engines; all the
                        # writeback DMA rides the (otherwise idle) sync queue
                        # so gpsimd/scalar can drain during o_proj
                        if (mo + si) % 2 == 0:
                            nc.scalar.activation(y_sb[:], ps_y[:], ActF.Copy)
                        else:
                            nc.vector.tensor_copy(y_sb[:], ps_y[:])
                        nc.sync.dma_start(dst, y_sb[:])
                if with_collective:
                    # core c receives row-block mo = chunk*8 + c
                    nc.gpsimd.collective_compute(
                        "ReduceScatter",
                        Alu.add,
                        replica_groups=[list(range(NCORES))],
                        ins=[cc_in[chunk * MO_PER:(chunk + 1) * MO_PER].opt()],
                        outs=[cc_out[chunk:chunk + 1].opt()],
                    )
                    nc.sync.dma_start(yout[chunk:chunk + 1],
                                      cc_out[chunk:chunk + 1])
                else:
                    # profiling-only variant: per-chunk local copy instead of
                    # the collective (overlaps with the next chunk's compute,
                    # mirroring the chunked ReduceScatter; output is the
                    # unreduced local shard)
                    nc.sync.dma_start(yout[chunk:chunk + 1],
                                      cc_in[chunk * MO_PER:chunk * MO_PER + 1])

    nc.compile()
    return nc


def _rot_matrix():
    # q_rot = R @ q with rotate_half along D: R @ v = concat(-v[64:], v[:64])
    R = np.zeros((128, 128), np.float32)
    for i in range(64):
        R[i, 64 + i] = -1.0
        R[64 + i, i] = 1.0
    return R


def _bf(a):
    return np.ascontiguousarray(a).astype(NP_BF16)


def _prep_in_maps(inputs):
    x = np.ascontiguousarray(np.asarray(inputs["hidden_states"],
                                        np.float32)[0, :, 0, :])
    wq = np.asarray(inputs["wq"], np.float32)
    wk = np.asarray(inputs["wk"], np.float32)
    wv = np.asarray(inputs["wv"], np.float32)
    wo = np.asarray(inputs["wo"], np.float32)
    bq = np.asarray(inputs["bq"], np.float32)
    bk = np.asarray(inputs["bk"], np.float32)
    bv = np.asarray(inputs["bv"], np.float32)
    cos_t = _bf(np.asarray(inputs["cos_t"], np.float32)[0, 0])  # (128, S)
    sin_t = _bf(np.asarray(inputs["sin_t"], np.float32)[0, 0])
    rotT = _bf(_rot_matrix().T)

    x_r = _bf(x.reshape(KO, 128, S))
    in_maps = []
    for c in range(NCORES):
        qs = slice(c * G * 128, (c + 1) * G * 128)
        ks = slice(c * 128, (c + 1) * 128)
        # wq -> (d, ko, m): wqT[ko*128+d, m]
        wq_t = _bf(wq[qs].T.reshape(KO, 128, G * 128).transpose(1, 0, 2))
        wk_t = _bf(wk[ks].T.reshape(KO, 128, 128).transpose(1, 0, 2))
        wv_t = _bf(wv[ks].T.reshape(KO, 128, 128).transpose(1, 0, 2))
        # wo column shard -> (d, mo, g, m): woT[g*128+d, mo*128+m]
        wo_t = _bf(wo[:, qs].T.reshape(G, 128, MO, 128).transpose(1, 2, 0, 3))
        in_maps.append({
            "x": x_r,
            "wq": wq_t,
            "wk": wk_t,
            "wv": wv_t,
            "wo": wo_t,
            "bq": np.ascontiguousarray(bq[qs].reshape(G, 128).T),
            "bk": np.ascontiguousarray(bk[ks][:, None]),
            "bv": np.ascontiguousarray(bv[ks][:, None]),
            "cos": cos_t,
            "sin": sin_t,
            "rot": rotT,
        })
    return in_maps


_NC = None


def _get_nc():
    global _NC
    if _NC is None:
        _NC = build_nc()
    return _NC


def assemble_output(results):
    """Chunked ReduceScatter: core c's chunk i is y row-block mo = 8*i + c."""
    y = np.empty((HID, S), np.float32)
    for c in range(NCORES):
        yc = np.asarray(results[c]["y"], np.float32)
        for i in range(yc.shape[0]):
            mo = NCORES * i + c
            y[mo * 128:(mo + 1) * 128] = yc[i]
    return y[None, :, None, :]


def kernel(**inputs):
    nc = _get_nc()
    in_maps = _prep_in_maps(inputs)
    res = run_bass_kernel_spmd(nc, in_maps, core_ids=list(range(NCORES)))
    return assemble_output(res.results)


# revision 17
# speedup vs baseline: 1.0677x; 1.0677x over previous
"""Trainium2 Bass kernel for AttentionPatcher (GQA attention block, S=2048).

Sharding: 8-way tensor parallel over KV head groups. Core c owns KV head c
and query heads 4c..4c+3: it computes its Q/K/V projections, RoPE, causal
attention, and a full partial o_proj (wo column shard); a ReduceScatter(add)
over the 8 cores then leaves core c with rows [512c, 512c+512) of the final
output, which the host concatenates.

All matmul operands are bf16 (fp32 PSUM accumulation). The PE only reaches
its top p-state after ~3us of continuous execution, so the kernel is
structured to keep the PE queue dense: all weights are SBUF-resident (wq/wo
loaded once up front on otherwise-idle DMA queues), and the attention inner
loop is software-pipelined — scores for pair i+1 issue before the AV/den
matmuls of pair i, so the PE never waits on the exp/mask chain.
"""
import os
import sys

import numpy as np
import ml_dtypes

if os.path.isdir("/opt/trn_rl_repo") and "/opt/trn_rl_repo" not in sys.path:
    sys.path.insert(0, "/opt/trn_rl_repo")

import concourse.bacc as bacc
import concourse.mybir as mybir
import concourse.tile as tile
from concourse.bass_utils import run_bass_kernel_spmd
from concourse.masks import make_identity

F32 = mybir.dt.float32
BF16 = mybir.dt.bfloat16
ActF = mybir.ActivationFunctionType
Alu = mybir.AluOpType
NP_BF16 = ml_dtypes.bfloat16

H, KV, D, S = 32, 8, 128, 2048
HID = H * D
NCORES = 8
G = H // KV          # query heads per core
ST = 512             # s-tile size
NST = S // ST        # 4 s-tiles
KO = HID // 128      # 32 contraction subtiles
MO = HID // 128      # 32 output row tiles
INV_SQRT_D = 1.0 / float(np.sqrt(D))


def build_nc(with_collective=True):
    nc = bacc.Bacc("TRN2", target_bir_lowering=False, debug=False)

    x = nc.dram_tensor("x", [KO, 128, S], BF16, kind="ExternalInput")
    wq = nc.dram_tensor("wq", [128, KO, G * 128], BF16, kind="ExternalInput")
    wk = nc.dram_tensor("wk", [128, KO, 128], BF16, kind="ExternalInput")
    wv = nc.dram_tensor("wv", [128, KO, 128], BF16, kind="ExternalInput")
    wo = nc.dram_tensor("wo", [128, MO, G, 128], BF16, kind="ExternalInput")
    bq = nc.dram_tensor("bq", [128, G], F32, kind="ExternalInput")
    bk = nc.dram_tensor("bk", [128, 1], F32, kind="ExternalInput")
    bv = nc.dram_tensor("bv", [128, 1], F32, kind="ExternalInput")
    cos = nc.dram_tensor("cos", [128, S], BF16, kind="ExternalInput")
    sin = nc.dram_tensor("sin", [128, S], BF16, kind="ExternalInput")
    rot = nc.dram_tensor("rot", [128, 128], BF16, kind="ExternalInput")
    yout = nc.dram_tensor("y", [G, 128, S], BF16, kind="ExternalOutput")

    with tile.TileContext(nc) as tc:
        with (
            tc.tile_pool(name="const", bufs=1) as const,
            tc.tile_pool(name="sb", bufs=3) as sb,
            tc.tile_pool(name="ps", bufs=1, space="PSUM") as ps,
            tc.tile_pool(name="dram", bufs=1, space="DRAM") as dram,
        ):
            # ---- resident weights / constants ----
            # wq/wk/wv go through the scalar+gpsimd DMA queues (chunked,
            # interleaved so early ko chunks land first) so the sync queue
            # starts streaming x immediately; wo is loaded later on the sync
            # queue, interleaved with si=1/2 x tiles.
            wq_sb = const.tile([128, KO, G * 128], BF16)
            wk_sb = const.tile([128, KO, 128], BF16)
            wv_sb = const.tile([128, KO, 128], BF16)
            wo_sb = const.tile([128, MO, G, 128], BF16)
            # fine-grained (2-ko) chunks: the first QKV matmuls only wait on
            # a small transfer, and the weight stream stays ahead of the PE
            for c16 in range(16):
                ksl = slice(c16 * 2, (c16 + 1) * 2)
                nc.scalar.dma_start(wq_sb[:, ksl, :], wq[:, ksl, :])
                nc.gpsimd.dma_start(wk_sb[:, ksl, :], wk[:, ksl, :])
                nc.gpsimd.dma_start(wv_sb[:, ksl, :], wv[:, ksl, :])
            cos_sb = const.tile([128, S], BF16)
            sin_sb = const.tile([128, S], BF16)
            nc.scalar.dma_start(cos_sb[:], cos[:, :])
            nc.gpsimd.dma_start(sin_sb[:], sin[:, :])
            rot_sb = const.tile([128, 128], BF16)
            nc.scalar.dma_start(rot_sb[:], rot[:, :])
            bq_sb = const.tile([128, G], F32)
            bk_sb = const.tile([128, 1], F32)
            bv_sb = const.tile([128, 1], F32)
            nc.scalar.dma_start(bq_sb[:], bq[:, :])
            nc.scalar.dma_start(bk_sb[:], bk[:, :])
            nc.scalar.dma_start(bv_sb[:], bv[:, :])
            ones_f = const.tile([128, 128], F32)
            nc.any.memset(ones_f[:], 1.0)
            ones_r = const.tile([128, 128], BF16)
            nc.vector.tensor_copy(ones_r[:], ones_f[:])
            ident = const.tile([128, 128], F32)
            make_identity(nc, ident)

            # ---- resident activations ----
            k_rot = const.tile([128, S], BF16)          # K, (d, l) layout
            v_t = const.tile([128, S // 128, 128], BF16)  # V^T, (l%128, l//128, d)
            # attn out, one tile per (g, si) so o_proj deps stay per-slice
            out_t = [[const.tile([128, ST], BF16, name=f"out_{g}_{si}")
                      for si in range(NST)] for g in range(G)]

            def rope(raw_r, dst_ap, sl):
                """dst = raw*cos + (rot@raw)*sin over s-slice sl."""
                ps_rot = ps.tile([128, ST], F32, tag="mm", bufs=4, name="ps_rot")
                nc.tensor.matmul(ps_rot[:], rot_sb[:], raw_r[:],
                                 start=True, stop=True)
                t1 = sb.tile([128, ST], BF16, tag="rope_t1", bufs=2)
                t2 = sb.tile([128, ST], BF16, tag="rope_t2", bufs=2)
                nc.vector.tensor_tensor(t1[:], raw_r[:], cos_sb[:, sl], Alu.mult)
                nc.vector.tensor_tensor(t2[:], ps_rot[:], sin_sb[:, sl], Alu.mult)
                nc.vector.tensor_tensor(dst_ap, t1[:], t2[:], Alu.add)

            for si in range(NST):
                sl = slice(si * ST, (si + 1) * ST)
                # ---------- QKV projections for this s-tile ----------
                ps_q = [ps.tile([128, ST], F32, tag="mm", bufs=4, name=f"ps_q{g}")
                        for g in range(G)]
                ps_kv = ps.tile([128, 2 * ST], F32, tag="big", bufs=2)
                ps_k = ps_kv[:, 0:ST]
                ps_v = ps_kv[:, ST:2 * ST]
                # resident-wo chunk loads ride the scalar queue while the
                # scalar engine is idle (QKV of si=1/2, before exps start)
                if si in (1, 2):
                    for wc2 in range(4):
                        wc = (si - 1) * 4 + wc2
                        msl = slice(wc * (MO // 8), (wc + 1) * (MO // 8))
                        nc.scalar.dma_start(wo_sb[:, msl, :, :],
                                            wo[:, msl, :, :])
                for ko in range(KO):
                    xt = sb.tile([128, ST], BF16, tag="x", bufs=10)
                    nc.sync.dma_start(xt[:], x[ko][:, sl])
                    st = (ko == 0)
                    sp = (ko == KO - 1)
                    for g in range(G):
                        nc.tensor.matmul(ps_q[g][:],
                                         wq_sb[:, ko, g * 128:(g + 1) * 128],
                                         xt[:], start=st, stop=sp)
                    nc.tensor.matmul(ps_k, wk_sb[:, ko, :], xt[:],
                                     start=st, stop=sp)
                    nc.tensor.matmul(ps_v, wv_sb[:, ko, :], xt[:],
                                     start=st, stop=sp)

                nli = (si + 1) * (ST // 128)  # visible l-blocks
                npair = nli // 2

                # ---------- biases + rope, g0 chain first ----------
                # DVE order is chosen so q_rot[0] (which gates the first
                # scores matmul) is ready as early as possible; V work is
                # PE filler that overlaps the first exp.
                q_raws = []
                q_raw0 = sb.tile([128, ST], BF16, tag="q_raw", bufs=2,
                                 name="q_raw0")
                nc.vector.tensor_scalar(q_raw0[:], ps_q[0][:], bq_sb[:, 0:1],
                                        INV_SQRT_D, Alu.add, Alu.mult)
                v_sb = sb.tile([128, ST], F32, tag="v_sb", bufs=2)
                nc.vector.tensor_scalar(v_sb[:], ps_v, bv_sb[:, 0:1], None,
                                        Alu.add)
                q_rots = []
                q_rot0 = sb.tile([128, ST], BF16, tag="q_rot", bufs=4,
                                 name="q_rot0")
                rope(q_raw0, q_rot0[:], sl)
                q_rots.append(q_rot0)
                q_raw1 = sb.tile([128, ST], BF16, tag="q_raw", bufs=2,
                                 name="q_raw1")
                nc.vector.tensor_scalar(q_raw1[:], ps_q[1][:], bq_sb[:, 1:2],
                                        INV_SQRT_D, Alu.add, Alu.mult)
                q_rot1 = sb.tile([128, ST], BF16, tag="q_rot", bufs=4,
                                 name="q_rot1")
                rope(q_raw1, q_rot1[:], sl)
                q_rots.append(q_rot1)
                # V transpose into v_t: PE filler while the DVE works
                # through the rope chains
                for j in range(ST // 128):
                    ps_t = ps.tile([128, 128], F32, tag="mm", bufs=4)
                    nc.tensor.transpose(ps_t[:], v_sb[:, j * 128:(j + 1) * 128],
                                        ident[:])
                    nc.vector.tensor_copy(v_t[:, si * (ST // 128) + j, :],
                                          ps_t[:])
                k_raw = sb.tile([128, ST], BF16, tag="k_raw", bufs=2)
                nc.vector.tensor_scalar(k_raw[:], ps_k, bk_sb[:, 0:1], None,
                                        Alu.add)
                rope(k_raw, k_rot[:, sl], sl)
                for g in range(2, G):
                    q_raw = sb.tile([128, ST], BF16, tag="q_raw", bufs=2,
                                    name=f"q_raw{g}")
                    nc.vector.tensor_scalar(q_raw[:], ps_q[g][:],
                                            bq_sb[:, g:g + 1], INV_SQRT_D,
                                            Alu.add, Alu.mult)
                    q_rot = sb.tile([128, ST], BF16, tag="q_rot", bufs=4,
                                    name=f"q_rot{g}")
                    rope(q_raw, q_rot[:], sl)
                    q_rots.append(q_rot)

                # ---------- software-pipelined attention ----------
                jobs = [(g, pi) for g in range(G) for pi in range(npair)]

                def issue_scores(job):
                    g, pi = job
                    ps_s2 = ps.tile([128, 2 * ST], F32, tag="big", bufs=2)
                    offs = []
                    for h in range(2):
                        li = 2 * pi + h
                        j = li - si * (ST // 128)
                        # diagonal block j: columns [0, 128j) are fully
                        # masked -> skip them entirely
                        off = 128 * j if j > 0 else 0
                        offs.append(off)
                        nc.tensor.matmul(
                            ps_s2[:, h * ST + off:(h + 1) * ST],
                            k_rot[:, li * 128:(li + 1) * 128],
                            q_rots[g][:, off:], start=True, stop=True)
                    return ps_s2, offs

                cur = issue_scores(jobs[0])

                ps_av = {}
                ps_den = {}
                for idx, (g, pi) in enumerate(jobs):
                    ps_s2, offs = cur
                    p2 = sb.tile([128, 2 * ST], BF16, tag="p", bufs=4)
                    if offs == [0, 0]:
                        # off-diagonal pair: one contiguous exp (fewer
                        # instruction + semaphore hops on the scalar engine)
                        nc.scalar.activation(p2[:], ps_s2[:], ActF.Exp)
                    else:
                        for h in range(2):
                            off = offs[h]
                            nc.scalar.activation(
                                p2[:, h * ST + off:(h + 1) * ST],
                                ps_s2[:, h * ST + off:(h + 1) * ST],
                                ActF.Exp)
                    # keep the PE streaming: scores for the next pair go in
                    # front of this pair's AV/den in the PE queue
                    if idx + 1 < len(jobs):
                        nxt = issue_scores(jobs[idx + 1])
                    for h in range(2):
                        li = 2 * pi + h
                        j = li - si * (ST // 128)
                        if j >= 0:
                            off = offs[h]
                            # causal: keep where l <= s within the block
                            nc.gpsimd.affine_select(
                                out=p2[:, h * ST + off:(h + 1) * ST],
                                in_=p2[:, h * ST + off:(h + 1) * ST],
                                compare_op=Alu.is_ge, fill=0.0,
                                base=0, channel_multiplier=-1,
                                pattern=[[1, ST - off]],
                            )
                    if pi == 0:
                        ps_av[g] = ps.tile([128, ST], F32, tag="mm", bufs=4,
                                           name=f"ps_av{g}")
                        ps_den[g] = ps.tile([128, ST], F32, tag="mm", bufs=4,
                                            name=f"ps_den{g}")
                    for h in range(2):
                        li = 2 * pi + h
                        off = offs[h]
                        ph = p2[:, h * ST + off:(h + 1) * ST]
                        nc.tensor.matmul(ps_av[g][:, off:], v_t[:, li, :], ph,
                                         start=(li == 0),
                                         stop=(li == nli - 1))
                        nc.tensor.matmul(ps_den[g][:, off:], ones_r[:], ph,
                                         start=(li == 0),
                                         stop=(li == nli - 1))
                    if idx + 1 < len(jobs):
                        cur = nxt
                    if pi == npair - 1:
                        # evict accumulators fast so their PSUM slots
                        # recycle; the (fast-approx) reciprocal then runs
                        # off the critical path
                        den_sb = sb.tile([128, ST], F32, tag="den_sb", bufs=2)
                        nc.vector.tensor_copy(den_sb[:], ps_den[g][:])
                        av_sb = sb.tile([128, ST], F32, tag="av_sb", bufs=2)
                        nc.scalar.activation(av_sb[:], ps_av[g][:], ActF.Copy)
                        recip = sb.tile([128, ST], F32, tag="recip", bufs=2)
                        nc.vector.reciprocal_approx_fast(recip[:], den_sb[:])
                        # normalize on the (otherwise idle) gpsimd engine so
                        # the DVE queue drains fast — the next s-tile's QKV
                        # psum slots wait on these evictions
                        nc.gpsimd.tensor_tensor(out_t[g][si][:], av_sb[:],
                                                recip[:], Alu.mult)

            # ---------- o_proj: y_partial = wo_colshard @ out ----------
            # chunked: after each group of 8 row-blocks, ReduceScatter that
            # chunk (overlaps the collective with the next group's compute)
            NCHUNK = G  # 4 chunks of 8 row-blocks
            MO_PER = MO // NCHUNK
            cc_in = dram.tile([MO, 128, S], BF16)
            cc_out = dram.tile([NCHUNK, 128, S], BF16)
            for chunk in range(NCHUNK):
                for mo in range(chunk * MO_PER, (chunk + 1) * MO_PER):
                    for si in range(NST):
                        ps_y = ps.tile([128, ST], F32, tag="mm", bufs=4)
                        for g in range(G):
                            nc.tensor.matmul(ps_y[:], wo_sb[:, mo, g, :],
                                             out_t[g][si][:],
                                             start=(g == 0), stop=(g == G - 1))
                        y_sb = sb.tile([128, ST], BF16, tag="y_sb", bufs=8)
                        dst = cc_in[mo][:, si * ST:(si + 1) * ST]
                        # evictions alternate scalar/vector engines; all the
                        # writeback DMA rides the (otherwise idle) sync queue
                        # so gpsimd/scalar can drain during o_proj
                        if (mo + si) % 2 == 0:
                            nc.scalar.activation(y_sb[:], ps_y[:], ActF.Copy)
                        else:
                            nc.vector.tensor_copy(y_sb[:], ps_y[:])
                        nc.sync.dma_start(dst, y_sb[:])
                if with_collective:
                    # core c receives row-block mo = chunk*8 + c
                    nc.gpsimd.collective_compute(
                        "ReduceScatter",
                        Alu.add,
                        replica_groups=[list(range(NCORES))],
                        ins=[cc_in[chunk * MO_PER:(chunk + 1) * MO_PER].opt()],
                        outs=[cc_out[chunk:chunk + 1].opt()],
                    )
                    nc.sync.dma_start(yout[chunk:chunk + 1],
                                      cc_out[chunk:chunk + 1])
                else:
                    # profiling-only variant: per-chunk local copy instead of
                    # the collective (overlaps with the next chunk's compute,
                    # mirroring the chunked ReduceScatter; output is the
                    # unreduced local shard)
                    nc.sync.dma_start(yout[chunk:chunk + 1],
                                      cc_in[chunk * MO_PER:chunk * MO_PER + 1])

    nc.compile()
    return nc


def _rot_matrix():
    # q_rot = R @ q with rotate_half along D: R @ v = concat(-v[64:], v[:64])
    R = np.zeros((128, 128), np.float32)
    for i in range(64):
        R[i, 64 + i] = -1.0
        R[64 + i, i] = 1.0
    return R


def _bf(a):
    return np.ascontiguousarray(a).astype(NP_BF16)


def _prep_in_maps(inputs):
    x = np.ascontiguousarray(np.asarray(inputs["hidden_states"],
                                        np.float32)[0, :, 0, :])
    wq = np.asarray(inputs["wq"], np.float32)
    wk = np.asarray(inputs["wk"], np.float32)
    wv = np.asarray(inputs["wv"], np.float32)
    wo = np.asarray(inputs["wo"], np.float32)
    bq = np.asarray(inputs["bq"], np.float32)
    bk = np.asarray(inputs["bk"], np.float32)
    bv = np.asarray(inputs["bv"], np.float32)
    cos_t = _bf(np.asarray(inputs["cos_t"], np.float32)[0, 0])  # (128, S)
    sin_t = _bf(np.asarray(inputs["sin_t"], np.float32)[0, 0])
    rotT = _bf(_rot_matrix().T)

    x_r = _bf(x.reshape(KO, 128, S))
    in_maps = []
    for c in range(NCORES):
        qs = slice(c * G * 128, (c + 1) * G * 128)
        ks = slice(c * 128, (c + 1) * 128)
        # wq -> (d, ko, m): wqT[ko*128+d, m]
        wq_t = _bf(wq[qs].T.reshape(KO, 128, G * 128).transpose(1, 0, 2))
        wk_t = _bf(wk[ks].T.reshape(KO, 128, 128).transpose(1, 0, 2))
        wv_t = _bf(wv[ks].T.reshape(KO, 128, 128).transpose(1, 0, 2))
        # wo column shard -> (d, mo, g, m): woT[g*128+d, mo*128+m]
        wo_t = _bf(wo[:, qs].T.reshape(G, 128, MO, 128).transpose(1, 2, 0, 3))
        in_maps.append({
            "x": x_r,
            "wq": wq_t,
            "wk": wk_t,
            "wv": wv_t,
            "wo": wo_t,
            "bq": np.ascontiguousarray(bq[qs].reshape(G, 128).T),
            "bk": np.ascontiguousarray(bk[ks][:, None]),
            "bv": np.ascontiguousarray(bv[ks][:, None]),
            "cos": cos_t,
            "sin": sin_t,
            "rot": rotT,
        })
    return in_maps


_NC = None


def _get_nc():
    global _NC
    if _NC is None:
        _NC = build_nc()
    return _NC


def assemble_output(results):
    """Chunked ReduceScatter: core c's chunk i is y row-block mo = 8*i + c."""
    y = np.empty((HID, S), np.float32)
    for c in range(NCORES):
        yc = np.asarray(results[c]["y"], np.float32)
        for i in range(yc.shape[0]):
            mo = NCORES * i + c
            y[mo * 128:(mo + 1) * 128] = yc[i]
    return y[None, :, None, :]


def kernel(**inputs):
    nc = _get_nc()
    in_maps = _prep_in_maps(inputs)
    res = run_bass_kernel_spmd(nc, in_maps, core_ids=list(range(NCORES)))
    return assemble_output(res.results)


# revision 23
# speedup vs baseline: 1.2777x; 1.1967x over previous
"""Trainium2 Bass kernel for AttentionPatcher (GQA attention block, S=2048).

Sharding: 8-way tensor parallel over KV head groups. Core c owns KV head c
and query heads 4c..4c+3: it computes its Q/K/V projections, RoPE, causal
attention, and a full partial o_proj (wo column shard); a ReduceScatter(add)
over the 8 cores then leaves core c with rows [512c, 512c+512) of the final
output, which the host concatenates.

All matmul operands are bf16 (fp32 PSUM accumulation). The PE only reaches
its top p-state after ~3us of continuous execution, so the kernel is
structured to keep the PE queue dense: all weights are SBUF-resident (wq/wo
loaded once up front on otherwise-idle DMA queues), and the attention inner
loop is software-pipelined — scores for pair i+1 issue before the AV/den
matmuls of pair i, so the PE never waits on the exp/mask chain.
"""
import os
import sys

import numpy as np
import ml_dtypes

if os.path.isdir("/opt/trn_rl_repo") and "/opt/trn_rl_repo" not in sys.path:
    sys.path.insert(0, "/opt/trn_rl_repo")

import concourse.bacc as bacc
import concourse.mybir as mybir
import concourse.tile as tile
from concourse.bass_utils import run_bass_kernel_spmd
from concourse.masks import make_identity

F32 = mybir.dt.float32
BF16 = mybir.dt.bfloat16
ActF = mybir.ActivationFunctionType
Alu = mybir.AluOpType
NP_BF16 = ml_dtypes.bfloat16

H, KV, D, S = 32, 8, 128, 2048
HID = H * D
NCORES = 8
G = H // KV          # query heads per core
ST = 512             # s-tile size
NST = S // ST        # 4 s-tiles
KO = HID // 128      # 32 contraction subtiles
MO = HID // 128      # 32 output row tiles
INV_SQRT_D = 1.0 / float(np.sqrt(D))


def build_nc(with_collective=True):
    nc = bacc.Bacc("TRN2", target_bir_lowering=False, debug=False)

    x = nc.dram_tensor("x", [KO, 128, S], BF16, kind="ExternalInput")
    wq = nc.dram_tensor("wq", [128, KO, G * 128], BF16, kind="ExternalInput")
    wk = nc.dram_tensor("wk", [128, KO, 128], BF16, kind="ExternalInput")
    wv = nc.dram_tensor("wv", [128, KO, 128], BF16, kind="ExternalInput")
    wo = nc.dram_tensor("wo", [128, MO, G, 128], BF16, kind="ExternalInput")
    bq = nc.dram_tensor("bq", [128, G], F32, kind="ExternalInput")
    bk = nc.dram_tensor("bk", [128, 1], F32, kind="ExternalInput")
    bv = nc.dram_tensor("bv", [128, 1], F32, kind="ExternalInput")
    cos = nc.dram_tensor("cos", [128, S], BF16, kind="ExternalInput")
    sin = nc.dram_tensor("sin", [128, S], BF16, kind="ExternalInput")
    rot = nc.dram_tensor("rot", [128, 128], BF16, kind="ExternalInput")
    yout = nc.dram_tensor("y", [G, 128, S], BF16, kind="ExternalOutput")

    with tile.TileContext(nc) as tc:
        with (
            tc.tile_pool(name="const", bufs=1) as const,
            tc.tile_pool(name="sb", bufs=3) as sb,
            tc.tile_pool(name="ps", bufs=1, space="PSUM") as ps,
            tc.tile_pool(name="dram", bufs=1, space="DRAM") as dram,
        ):
            # ---- resident weights / constants ----
            # wq/wk/wv go through the scalar+gpsimd DMA queues (chunked,
            # interleaved so early ko chunks land first) so the sync queue
            # starts streaming x immediately; wo is loaded later on the sync
            # queue, interleaved with si=1/2 x tiles.
            wq_sb = const.tile([128, KO, G * 128], BF16)
            wk_sb = const.tile([128, KO, 128], BF16)
            wv_sb = const.tile([128, KO, 128], BF16)
            wo_sb = const.tile([128, MO, G, 128], BF16)
            # fine-grained (2-ko) chunks: the first QKV matmuls only wait on
            # a small transfer, and the weight stream stays ahead of the PE
            for c16 in range(16):
                ksl = slice(c16 * 2, (c16 + 1) * 2)
                nc.scalar.dma_start(wq_sb[:, ksl, :], wq[:, ksl, :])
                nc.gpsimd.dma_start(wk_sb[:, ksl, :], wk[:, ksl, :])
                nc.gpsimd.dma_start(wv_sb[:, ksl, :], wv[:, ksl, :])
            cos_sb = const.tile([128, S], BF16)
            sin_sb = const.tile([128, S], BF16)
            nc.scalar.dma_start(cos_sb[:], cos[:, :])
            nc.gpsimd.dma_start(sin_sb[:], sin[:, :])
            rot_sb = const.tile([128, 128], BF16)
            nc.scalar.dma_start(rot_sb[:], rot[:, :])
            bq_sb = const.tile([128, G], F32)
            bk_sb = const.tile([128, 1], F32)
            bv_sb = const.tile([128, 1], F32)
            nc.scalar.dma_start(bq_sb[:], bq[:, :])
            nc.scalar.dma_start(bk_sb[:], bk[:, :])
            nc.scalar.dma_start(bv_sb[:], bv[:, :])
            ones_f = const.tile([128, 128], F32)
            nc.any.memset(ones_f[:], 1.0)
            ones_r = const.tile([128, 128], BF16)
            nc.vector.tensor_copy(ones_r[:], ones_f[:])
            ident = const.tile([128, 128], BF16)
            make_identity(nc, ident)
            # causal triangle mask: mask[l, s] = 1 if s >= l else 0.
            # After off-slicing, every diagonal block keeps exactly the
            # region ds_rel >= dl, so one resident tile serves them all.
            mask_sb = const.tile([128, ST], BF16)
            nc.gpsimd.memset(mask_sb[:], 1.0)
            nc.gpsimd.affine_select(
                out=mask_sb[:], in_=mask_sb[:],
                compare_op=Alu.is_ge, fill=0.0,
                base=0, channel_multiplier=-1, pattern=[[1, ST]],
            )

            # ---- resident activations ----
            k_rot = const.tile([128, S], BF16)          # K, (d, l) layout
            v_t = const.tile([128, S // 128, 128], BF16)  # V^T, (l%128, l//128, d)
            # attn out, one tile per (g, si) so o_proj deps stay per-slice
            out_t = [[const.tile([128, ST], BF16, name=f"out_{g}_{si}")
                      for si in range(NST)] for g in range(G)]

            def rope(raw_r, dst_ap, sl):
                """dst = raw*cos + (rot@raw)*sin over s-slice sl."""
                ps_rot = ps.tile([128, ST], F32, tag="mm", bufs=4, name="ps_rot")
                nc.tensor.matmul(ps_rot[:], rot_sb[:], raw_r[:],
                                 start=True, stop=True)
                t1 = sb.tile([128, ST], BF16, tag="rope_t1", bufs=2)
                t2 = sb.tile([128, ST], BF16, tag="rope_t2", bufs=2)
                nc.vector.tensor_tensor(t1[:], raw_r[:], cos_sb[:, sl], Alu.mult)
                nc.vector.tensor_tensor(t2[:], ps_rot[:], sin_sb[:, sl], Alu.mult)
                nc.vector.tensor_tensor(dst_ap, t1[:], t2[:], Alu.add)

            for si in range(NST):
                sl = slice(si * ST, (si + 1) * ST)
                # ---------- QKV projections for this s-tile ----------
                ps_q = [ps.tile([128, ST], F32, tag="mm", bufs=4, name=f"ps_q{g}")
                        for g in range(G)]
                ps_kv = ps.tile([128, 2 * ST], F32, tag="big", bufs=2)
                ps_k = ps_kv[:, 0:ST]
                ps_v = ps_kv[:, ST:2 * ST]
                # resident-wo chunk loads ride the scalar queue while the
                # scalar engine is idle (QKV of si=1/2, before exps start)
                if si in (1, 2):
                    for wc2 in range(4):
                        wc = (si - 1) * 4 + wc2
                        msl = slice(wc * (MO // 8), (wc + 1) * (MO // 8))
                        nc.scalar.dma_start(wo_sb[:, msl, :, :],
                                            wo[:, msl, :, :])
                for ko in range(KO):
                    xt = sb.tile([128, ST], BF16, tag="x", bufs=10)
                    nc.sync.dma_start(xt[:], x[ko][:, sl])
                    st = (ko == 0)
                    sp = (ko == KO - 1)
                    for g in range(G):
                        nc.tensor.matmul(ps_q[g][:],
                                         wq_sb[:, ko, g * 128:(g + 1) * 128],
                                         xt[:], start=st, stop=sp)
                    nc.tensor.matmul(ps_k, wk_sb[:, ko, :], xt[:],
                                     start=st, stop=sp)
                    nc.tensor.matmul(ps_v, wv_sb[:, ko, :], xt[:],
                                     start=st, stop=sp)

                nli = (si + 1) * (ST // 128)  # visible l-blocks
                npair = nli // 2

                # ---------- biases + rope, g0 chain first ----------
                # DVE order is chosen so q_rot[0] (which gates the first
                # scores matmul) is ready as early as possible; V work is
                # PE filler that overlaps the first exp.
                q_raws = []
                q_raw0 = sb.tile([128, ST], BF16, tag="q_raw", bufs=2,
                                 name="q_raw0")
                nc.vector.tensor_scalar(q_raw0[:], ps_q[0][:], bq_sb[:, 0:1],
                                        INV_SQRT_D, Alu.add, Alu.mult)
                v_sb = sb.tile([128, ST], BF16, tag="v_sb", bufs=2)
                nc.vector.tensor_scalar(v_sb[:], ps_v, bv_sb[:, 0:1], None,
                                        Alu.add)
                q_rots = []
                q_rot0 = sb.tile([128, ST], BF16, tag="q_rot", bufs=4,
                                 name="q_rot0")
                rope(q_raw0, q_rot0[:], sl)
                q_rots.append(q_rot0)
                q_raw1 = sb.tile([128, ST], BF16, tag="q_raw", bufs=2,
                                 name="q_raw1")
                nc.vector.tensor_scalar(q_raw1[:], ps_q[1][:], bq_sb[:, 1:2],
                                        INV_SQRT_D, Alu.add, Alu.mult)
                q_rot1 = sb.tile([128, ST], BF16, tag="q_rot", bufs=4,
                                 name="q_rot1")
                rope(q_raw1, q_rot1[:], sl)
                q_rots.append(q_rot1)
                # V transpose into v_t: PE filler while the DVE works
                # through the rope chains
                for j in range(ST // 128):
                    ps_t = ps.tile([128, 128], BF16, tag="mm", bufs=4)
                    nc.tensor.transpose(ps_t[:], v_sb[:, j * 128:(j + 1) * 128],
                                        ident[:])
                    nc.vector.tensor_copy(v_t[:, si * (ST // 128) + j, :],
                                          ps_t[:])
                k_raw = sb.tile([128, ST], BF16, tag="k_raw", bufs=2)
                nc.vector.tensor_scalar(k_raw[:], ps_k, bk_sb[:, 0:1], None,
                                        Alu.add)
                rope(k_raw, k_rot[:, sl], sl)
                for g in range(2, G):
                    q_raw = sb.tile([128, ST], BF16, tag="q_raw", bufs=2,
                                    name=f"q_raw{g}")
                    nc.vector.tensor_scalar(q_raw[:], ps_q[g][:],
                                            bq_sb[:, g:g + 1], INV_SQRT_D,
                                            Alu.add, Alu.mult)
                    q_rot = sb.tile([128, ST], BF16, tag="q_rot", bufs=4,
                                    name=f"q_rot{g}")
                    rope(q_raw, q_rot[:], sl)
                    q_rots.append(q_rot)

                # ---------- software-pipelined attention ----------
                jobs = [(g, pi) for g in range(G) for pi in range(npair)]

                def issue_scores(job):
                    g, pi = job
                    ps_s2 = ps.tile([128, 2 * ST], F32, tag="big", bufs=2)
                    offs = []
                    for h in range(2):
                        li = 2 * pi + h
                        j = li - si * (ST // 128)
                        # diagonal block j: columns [0, 128j) are fully
                        # masked -> skip them entirely
                        off = 128 * j if j > 0 else 0
                        offs.append(off)
                        nc.tensor.matmul(
                            ps_s2[:, h * ST + off:(h + 1) * ST],
                            k_rot[:, li * 128:(li + 1) * 128],
                            q_rots[g][:, off:], start=True, stop=True)
                    return ps_s2, offs

                cur = issue_scores(jobs[0])

                ps_av = {}
                ps_den = {}
                for idx, (g, pi) in enumerate(jobs):
                    ps_s2, offs = cur
                    p2 = sb.tile([128, 2 * ST], BF16, tag="p", bufs=4)
                    if offs == [0, 0]:
                        # off-diagonal pair: one contiguous exp (fewer
                        # instruction + semaphore hops on the scalar engine)
                        nc.scalar.activation(p2[:], ps_s2[:], ActF.Exp)
                    else:
                        for h in range(2):
                            off = offs[h]
                            nc.scalar.activation(
                                p2[:, h * ST + off:(h + 1) * ST],
                                ps_s2[:, h * ST + off:(h + 1) * ST],
                                ActF.Exp)
                    # keep the PE streaming: scores for the next pair go in
                    # front of this pair's AV/den in the PE queue
                    if idx + 1 < len(jobs):
                        nxt = issue_scores(jobs[idx + 1])
                    for h in range(2):
                        li = 2 * pi + h
                        j = li - si * (ST // 128)
                        if j >= 0:
                            off = offs[h]
                            # causal: keep where l <= s within the block
                            # (multiplicative mask on the DVE — cheaper and
                            # lower-latency than a gpsimd affine_select)
                            nc.vector.tensor_tensor(
                                p2[:, h * ST + off:(h + 1) * ST],
                                p2[:, h * ST + off:(h + 1) * ST],
                                mask_sb[:, 0:ST - off], Alu.mult)
                    if pi == 0:
                        ps_av[g] = ps.tile([128, ST], F32, tag="mm", bufs=4,
                                           name=f"ps_av{g}")
                        ps_den[g] = ps.tile([128, ST], F32, tag="mm", bufs=4,
                                            name=f"ps_den{g}")
                    for h in range(2):
                        li = 2 * pi + h
                        off = offs[h]
                        ph = p2[:, h * ST + off:(h + 1) * ST]
                        nc.tensor.matmul(ps_av[g][:, off:], v_t[:, li, :], ph,
                                         start=(li == 0),
                                         stop=(li == nli - 1))
                        nc.tensor.matmul(ps_den[g][:, off:], ones_r[:], ph,
                                         start=(li == 0),
                                         stop=(li == nli - 1))
                    if idx + 1 < len(jobs):
                        cur = nxt
                    if pi == npair - 1:
                        # two-op head end, straight off PSUM: fewer total
                        # engine-ops keeps the core's power budget (and so
                        # the PE clock) up
                        recip = sb.tile([128, ST], F32, tag="recip", bufs=2)
                        nc.vector.reciprocal_approx_fast(recip[:],
                                                         ps_den[g][:])
                        nc.vector.tensor_tensor(out_t[g][si][:],
                                                ps_av[g][:], recip[:],
                                                Alu.mult)

            # ---------- o_proj: y_partial = wo_colshard @ out ----------
            # chunked: after each group of 8 row-blocks, ReduceScatter that
            # chunk (overlaps the collective with the next group's compute)
            NCHUNK = G  # 4 chunks of 8 row-blocks
            MO_PER = MO // NCHUNK
            cc_in = dram.tile([MO, 128, S], BF16)
            cc_out = dram.tile([NCHUNK, 128, S], BF16)
            for chunk in range(NCHUNK):
                for mo in range(chunk * MO_PER, (chunk + 1) * MO_PER):
                    for si in range(NST):
                        ps_y = ps.tile([128, ST], F32, tag="mm", bufs=4)
                        for g in range(G):
                            nc.tensor.matmul(ps_y[:], wo_sb[:, mo, g, :],
                                             out_t[g][si][:],
                                             start=(g == 0), stop=(g == G - 1))
                        y_sb = sb.tile([128, ST], BF16, tag="y_sb", bufs=8)
                        dst = cc_in[mo][:, si * ST:(si + 1) * ST]
                        # evictions alternate scalar/vector engines; all the
                        # writeback DMA rides the (otherwise idle) sync queue
                        # so gpsimd/scalar can drain during o_proj
                        if (mo + si) % 2 == 0:
                            nc.scalar.activation(y_sb[:], ps_y[:], ActF.Copy)
                        else:
                            nc.vector.tensor_copy(y_sb[:], ps_y[:])
                        nc.sync.dma_start(dst, y_sb[:])
                if with_collective:
                    # core c receives row-block mo = chunk*8 + c
                    nc.gpsimd.collective_compute(
                        "ReduceScatter",
                        Alu.add,
                        replica_groups=[list(range(NCORES))],
                        ins=[cc_in[chunk * MO_PER:(chunk + 1) * MO_PER].opt()],
                        outs=[cc_out[chunk:chunk + 1].opt()],
                    )
                    nc.sync.dma_start(yout[chunk:chunk + 1],
                                      cc_out[chunk:chunk + 1])
                else:
                    # profiling-only variant: per-chunk local copy instead of
                    # the collective (overlaps with the next chunk's compute,
                    # mirroring the chunked ReduceScatter; output is the
                    # unreduced local shard)
                    nc.sync.dma_start(yout[chunk:chunk + 1],
                                      cc_in[chunk * MO_PER:chunk * MO_PER + 1])

    nc.compile()
    return nc


def _rot_matrix():
    # q_rot = R @ q with rotate_half along D: R @ v = concat(-v[64:], v[:64])
    R = np.zeros((128, 128), np.float32)
    for i in range(64):
        R[i, 64 + i] = -1.0
        R[64 + i, i] = 1.0
    return R


def _bf(a):
    return np.ascontiguousarray(a).astype(NP_BF16)


def _prep_in_maps(inputs):
    x = np.ascontiguousarray(np.asarray(inputs["hidden_states"],
                                        np.float32)[0, :, 0, :])
    wq = np.asarray(inputs["wq"], np.float32)
    wk = np.asarray(inputs["wk"], np.float32)
    wv = np.asarray(inputs["wv"], np.float32)
    wo = np.asarray(inputs["wo"], np.float32)
    bq = np.asarray(inputs["bq"], np.float32)
    bk = np.asarray(inputs["bk"], np.float32)
    bv = np.asarray(inputs["bv"], np.float32)
    cos_t = _bf(np.asarray(inputs["cos_t"], np.float32)[0, 0])  # (128, S)
    sin_t = _bf(np.asarray(inputs["sin_t"], np.float32)[0, 0])
    rotT = _bf(_rot_matrix().T)

    x_r = _bf(x.reshape(KO, 128, S))
    in_maps = []
    for c in range(NCORES):
        qs = slice(c * G * 128, (c + 1) * G * 128)
        ks = slice(c * 128, (c + 1) * 128)
        # wq -> (d, ko, m): wqT[ko*128+d, m]
        wq_t = _bf(wq[qs].T.reshape(KO, 128, G * 128).transpose(1, 0, 2))
        wk_t = _bf(wk[ks].T.reshape(KO, 128, 128).transpose(1, 0, 2))
        wv_t = _bf(wv[ks].T.reshape(KO, 128, 128).transpose(1, 0, 2))
        # wo column shard -> (d, mo, g, m): woT[g*128+d, mo*128+m]
        wo_t = _bf(wo[:, qs].T.reshape(G, 128, MO, 128).transpose(1, 2, 0, 3))
        in_maps.append({
            "x": x_r,
            "wq": wq_t,
            "wk": wk_t,
            "wv": wv_t,
            "wo": wo_t,
            "bq": np.ascontiguousarray(bq[qs].reshape(G, 128).T),
            "bk": np.ascontiguousarray(bk[ks][:, None]),
            "bv": np.ascontiguousarray(bv[ks][:, None]),
            "cos": cos_t,
            "sin": sin_t,
            "rot": rotT,
        })
    return in_maps


_NC = None


def _get_nc():
    global _NC
    if _NC is None:
        _NC = build_nc()
    return _NC


def assemble_output(results):
    """Chunked ReduceScatter: core c's chunk i is y row-block mo = 8*i + c."""
    y = np.empty((HID, S), np.float32)
    for c in range(NCORES):
        yc = np.asarray(results[c]["y"], np.float32)
        for i in range(yc.shape[0]):
            mo = NCORES * i + c
            y[mo * 128:(mo + 1) * 128] = yc[i]
    return y[None, :, None, :]


def kernel(**inputs):
    nc = _get_nc()
    in_maps = _prep_in_maps(inputs)
    res = run_bass_kernel_spmd(nc, in_maps, core_ids=list(range(NCORES)))
    return assemble_output(res.results)
